# revision 18
# baseline (speedup 1.0000x reference)
"""Trainium2 Bass kernel for nn_EncoderLayer (GNN message passing, 2-relation GAT).

Sharding: nodes (and incoming-edge lists, partitioned by dst) sharded across 8
cores; small GAT/FFN weights replicated; gathered src features fetched from a
replicated projection table via indexed DMA (dma_gather).

Per-core device program:
  Phase 0: cast/fold weights (W_ext = [W | W.al | W.ar]) on device.
  Phase 1: BN1 + z/el/er projection for ALL nodes (replicated), packed rows
           written to DRAM: Zpack[node] = 12 x (64 z | 4 el) bf16, Er[node].
           BN rsqrt via scalar Sqrt + vector reciprocal (keeps scalar on one
           activation table for the whole phase).
  Phase 2: per dst-window (128 nodes): dma_gather Zpack[src] with a per-core
           runtime row count (tail indices are -1 and skipped); er[dst] comes
           from a 128-row window gather broadcast to edges via an S^T matmul;
           ex = exp(leaky(el+er)); msg = ex*z; segment-sum via one-hot
           S-matrix matmuls accumulating in PSUM (denominator columns fed
           straight from the ex tile); m = msgsum/denom; x2 = x + m1 + m2;
           BN2 stats stashed per window.
  Phase 3: batched BN2 scale computation (one Sqrt), then per window:
           h2 = BN2(x2); FFN via PE transposes; out = x2 + FFN.
"""

import sys

sys.path.insert(0, "/opt/trn_rl_repo")

import numpy as np
import ml_dtypes

import concourse.bass as bass
import concourse.bacc as bacc
import concourse.tile as tile
import concourse.mybir as mybir
from concourse.bass_utils import run_bass_kernel_spmd

F32 = mybir.dt.float32
BF16 = mybir.dt.bfloat16
I16 = mybir.dt.int16
I32 = mybir.dt.int32
AF = mybir.ActivationFunctionType
ALU = mybir.AluOpType
BF16NP = ml_dtypes.bfloat16

N, T, D, H, DH, DFF = 10000, 12, 64, 4, 16, 128
NCORES = 8
CHUNK = N // NCORES          # 1250
WIN = 128                    # dst-window size (nodes)
NW = (CHUNK + WIN - 1) // WIN  # 10 windows; last has 98 nodes
EPS = 1e-5
NEG_SLOPE = 0.2
ZROW = 896                   # padded Zpack row (bf16 elems): 12*68 data + 80 pad
ERROW = 128                  # padded Er row (bf16 elems): 48 data + 80 pad
NBLK = (N + 127) // 128      # 79 phase-1 blocks (last = 16 nodes)
DYN_GATHER = True            # runtime row counts (-1 padded) for zg gathers


def _win_nodes(w):
    return min(WIN, CHUNK - w * WIN)


def _wrap16(arr):
    """dma_gather index layout: idx i at [i%16, i//16], tiled to 128 parts."""
    return np.ascontiguousarray(np.tile(arr.reshape(-1, 16).T, (8, 1)))


def _prep_core_rel(src, dst, lo, B):
    """Edge lists for one (core, relation): sorted by dst, windowed, padded to
    B blocks of 128 edges per window with -1 (skipped by dma_gather).
    Returns (src16, cnts, S, ST) where cnts[w] is the real edge count and
    S[w, edge_in_block, block*128 + local_node] is the one-hot segment-sum
    stationary (ST its per-block transpose)."""
    hi = lo + CHUNK
    sel = (dst >= lo) & (dst < hi)
    es = src[sel].astype(np.int64)
    ed = (dst[sel] - lo).astype(np.int64)
    order = np.argsort(ed, kind="stable")
    es, ed = es[order], ed[order]
    L = NW * B * 128
    src_arr = np.full(L, -1 if DYN_GATHER else 0, np.int16)
    cnts = np.zeros(NW, np.int32)
    S = np.zeros((NW, 128, B * 128), BF16NP)
    ST = np.zeros((NW, 128, B * 128), BF16NP)
    wstart = np.searchsorted(ed, np.arange(NW) * WIN)
    wend = np.searchsorted(ed, np.arange(1, NW + 1) * WIN)
    for w in range(NW):
        seg_src = es[wstart[w]:wend[w]]
        seg_dst = ed[wstart[w]:wend[w]]
        cnt = len(seg_src)
        assert 0 < cnt <= B * 128
        base = w * B * 128
        src_arr[base:base + cnt] = seg_src
        cnts[w] = cnt
        i = np.arange(cnt)
        S[w, i % 128, (i // 128) * 128 + (seg_dst - w * WIN)] = 1.0
        ST[w, seg_dst - w * WIN, (i // 128) * 128 + (i % 128)] = 1.0
    return _wrap16(src_arr), cnts, S, ST


def _max_blocks(src, dst):
    """Max number of 128-edge blocks any (core, window) needs for this rel."""
    best = 0
    for m in range(NCORES):
        lo = m * CHUNK
        sel = (dst >= lo) & (dst < lo + CHUNK)
        ed = dst[sel] - lo
        cnt = np.bincount(ed // WIN, minlength=NW)
        best = max(best, int(np.max((cnt + 127) // 128)))
    return best


def _build_program(B, phases=3, dump=None):
    nc = bacc.Bacc("TRN2", target_bir_lowering=False, debug=False,
                   num_devices=NCORES)
    BL = B * 128              # edges per window (padded)
    L = NW * BL               # edges per (core, rel)
    L16 = L // 16

    # ---- DRAM tensors ----
    x_full = nc.dram_tensor("x_full", [N, T * D], F32, kind="ExternalInput")
    xc = nc.dram_tensor("xc", [CHUNK, T * D], F32, kind="ExternalInput")
    bn1_gb = nc.dram_tensor("bn1_gb", [N, 2], F32, kind="ExternalInput")
    bn2_gb = nc.dram_tensor("bn2_gb", [NW * 128, 2], F32, kind="ExternalInput")
    cnt_in = nc.dram_tensor("cnts", [1, 2 * NW], I32, kind="ExternalInput")
    dw_in = nc.dram_tensor("dstwin", [128, NW * 8], I16, kind="ExternalInput")
    w_in, al_in, ar_in, s_in, st_in, si_in = [], [], [], [], [], []
    for r in (1, 2):
        w_in.append(nc.dram_tensor(f"W{r}", [D, H * DH], F32, kind="ExternalInput"))
        al_in.append(nc.dram_tensor(f"al{r}t", [D, H * DH], F32, kind="ExternalInput"))
        ar_in.append(nc.dram_tensor(f"ar{r}t", [D, H * DH], F32, kind="ExternalInput"))
        s_in.append(nc.dram_tensor(f"S{r}", [NW, 128, BL], BF16, kind="ExternalInput"))
        st_in.append(nc.dram_tensor(f"ST{r}", [NW, 128, BL], BF16, kind="ExternalInput"))
        si_in.append(nc.dram_tensor(f"srcidx{r}", [128, L16], I16, kind="ExternalInput"))
    ffw1_in = nc.dram_tensor("ffw1", [D, DFF], F32, kind="ExternalInput")
    ffb1_in = nc.dram_tensor("ffb1", [DFF, 1], F32, kind="ExternalInput")
    ffw2_in = nc.dram_tensor("ffw2", [DFF, D], F32, kind="ExternalInput")
    ffb2_in = nc.dram_tensor("ffb2", [D, 1], F32, kind="ExternalInput")
    ident_in = nc.dram_tensor("ident", [128, 128], BF16, kind="ExternalInput")
    out_d = nc.dram_tensor("OUT", [CHUNK, T * D], F32, kind="ExternalOutput")

    zpack = [nc.dram_tensor(f"zpack{r}", [N, ZROW], BF16, kind="Internal")
             for r in (1, 2)]
    erd = [nc.dram_tensor(f"er{r}", [N, ERROW], BF16, kind="Internal")
           for r in (1, 2)]

    with tile.TileContext(nc) as tc:
        with tc.tile_pool(name="const", bufs=1) as cpool:
            # ---- Phase 0: weights to SBUF, fold al/ar into W_ext ----
            ident = cpool.tile([128, 128], BF16)
            nc.sync.dma_start(ident[:], ident_in[:])
            cnts = cpool.tile([1, 2 * NW], I32)
            nc.sync.dma_start(cnts[:], cnt_in[:])
            wext = []
            for r in range(2):
                wf = cpool.tile([D, H * DH], F32, tag="wf")
                nc.sync.dma_start(wf[:], w_in[r][:])
                we = cpool.tile([128, 72], BF16, tag=f"wext{r}")
                nc.vector.tensor_copy(we[0:D, 0:64], wf[:])
                for k, t_in in ((64, al_in[r]), (68, ar_in[r])):
                    alt = cpool.tile([D, H * DH], F32, tag="alt")
                    nc.sync.dma_start(alt[:], t_in[:])
                    prod = cpool.tile([D, H * DH], F32, tag="prod")
                    nc.vector.tensor_mul(prod[:], wf[:], alt[:])
                    red = cpool.tile([D, H], F32, tag="red")
                    nc.vector.tensor_reduce(
                        red[:].unsqueeze(2),
                        prod[:].rearrange("p (h k) -> p h k", k=DH),
                        mybir.AxisListType.X, ALU.add)
                    nc.vector.tensor_copy(we[0:D, k:k + H], red[:])
                wext.append(we)
            wboth = cpool.tile([64, 144], BF16)
            for r in range(2):
                nc.vector.tensor_copy(wboth[0:D, 72 * r:72 * r + 72],
                                      wext[r][0:D, :])
            ffw1 = cpool.tile([128, DFF], BF16)
            t1 = cpool.tile([D, DFF], F32, tag="t1")
            nc.sync.dma_start(t1[:], ffw1_in[:])
            nc.vector.tensor_copy(ffw1[0:D, :], t1[:])
            nc.sync.dma_start(ffw1[64:128, :], ffw1[0:64, :])
            ffw2 = cpool.tile([DFF, D], BF16)
            t2 = cpool.tile([DFF, D], F32, tag="t2")
            nc.sync.dma_start(t2[:], ffw2_in[:])
            nc.vector.tensor_copy(ffw2[:], t2[:])
            ffb1 = cpool.tile([DFF, 1], F32)
            nc.sync.dma_start(ffb1[:], ffb1_in[:])
            ffb2r = cpool.tile([128, 1], F32)
            nc.sync.dma_start(ffb2r[0:64, :], ffb2_in[:])
            nc.sync.dma_start(ffb2r[64:128, :], ffb2_in[:])
            epst = cpool.tile([128, 1], F32)
            nc.vector.memset(epst[:], EPS)
            # per-window BN2 stats/params stash (consumed batched in phase 3)
            mvall = cpool.tile([128, NW, 2], F32)
            gball = cpool.tile([128, NW, 2], F32)
            nc.sync.dma_start(
                gball[:],
                bn2_gb.ap().rearrange("(w p) c -> p w c", p=128))
            aball = cpool.tile([128, NW, 2], F32)

            if phases >= 1:
              # ---- Phase 1: BN1 + projections for all N nodes ----
              with (
                  tc.tile_pool(name="p1", bufs=3) as p1,
                  tc.tile_pool(name="p1s", bufs=3) as p1s,
                  tc.tile_pool(name="p1tp", bufs=1, space="PSUM") as p1tp,
                  tc.tile_pool(name="p1zp", bufs=1, space="PSUM") as p1zp,
              ):
                  for blk in range(NBLK):
                      n0 = blk * 128
                      nb = min(128, N - n0)
                      xt = p1.tile([128, T * D], F32, tag="xt")
                      nc.sync.dma_start(xt[:nb], x_full[n0:n0 + nb])
                      st6 = p1s.tile([128, 2, 6], F32, tag="st6")
                      nc.vector.bn_stats(st6[:nb, 0, :], xt[:nb, 0:384])
                      nc.vector.bn_stats(st6[:nb, 1, :], xt[:nb, 384:768])
                      mv = p1s.tile([128, 2], F32, tag="mv")
                      nc.vector.bn_aggr(mv[:nb], st6[:nb])
                      gb = p1s.tile([128, 2], F32, tag="gb")
                      nc.sync.dma_start(gb[:nb], bn1_gb[n0:n0 + nb])
                      rs = p1s.tile([128, 1], F32, tag="rs")
                      nc.scalar.activation(rs[:nb], mv[:nb, 1:2], AF.Sqrt,
                                           bias=epst[:nb])
                      nc.vector.reciprocal(rs[:nb], rs[:nb])
                      a = p1s.tile([128, 1], F32, tag="a")
                      nc.vector.tensor_mul(a[:nb], gb[:nb, 0:1], rs[:nb])
                      b = p1s.tile([128, 1], F32, tag="b")
                      nc.vector.tensor_mul(b[:nb], a[:nb], mv[:nb, 0:1])
                      nc.vector.tensor_sub(b[:nb], gb[:nb, 1:2], b[:nb])
                      h = p1.tile([128, T * D], BF16, tag="h")
                      nc.scalar.activation(h[:nb], xt[:nb], AF.Identity,
                                           bias=b[:nb], scale=a[:nb])
                      tp = p1tp.tile([64, T, 128], BF16, tag="tp")
                      for t in range(T):
                          nc.tensor.transpose(
                              tp[:, t, 0:nb], h[:nb, t * 64:(t + 1) * 64],
                              ident[:nb, :nb])
                      ht = p1.tile([64, T, 128], BF16, tag="ht")
                      nc.vector.tensor_copy(ht[:, :, 0:nb], tp[:, :, 0:nb])
                      zp = p1zp.tile([128, T, 256], F32, tag="zp")
                      for t in range(T):
                          nc.tensor.matmul(
                              zp[0:nb, t, 0:144], ht[:, t, 0:nb],
                              wboth[0:64, :], start=True, stop=True)
                      for r in range(2):
                          zel = p1.tile([128, ZROW], BF16, tag=f"zel{r}")
                          nc.scalar.activation(
                              zel[:nb, 0:T * 68].rearrange("p (t c) -> p t c", t=T),
                              zp[:nb, :, 72 * r:72 * r + 68], AF.Copy)
                          ers = p1s.tile([128, ERROW], BF16, tag=f"ers{r}")
                          nc.vector.tensor_copy(
                              ers[:nb, 0:T * H].rearrange("p (t c) -> p t c", t=T),
                              zp[:nb, :, 72 * r + 68:72 * r + 72])
                          nc.sync.dma_start(zpack[r][n0:n0 + nb, :], zel[:nb])
                          nc.sync.dma_start(erd[r][n0:n0 + nb, :], ers[:nb])

            tc.strict_bb_all_engine_barrier()

            if phases < 2:
                dummy = cpool.tile([128, 16], F32)
                nc.vector.memset(dummy[:], 0.0)
                for w in range(NW):
                    nw = _win_nodes(w)
                    nc.sync.dma_start(out_d[w * WIN:w * WIN + nw, 0:16],
                                      dummy[:nw])
            if phases >= 2:
              # ---- Phase 2: gather + attention + segment-sum + BN2 stats ----
              x2_tiles = []
              with (
                  tc.tile_pool(name="x2p", bufs=NW) as x2p,
              ):
                  with (
                      tc.tile_pool(name="idxp", bufs=1) as idxp,
                      tc.tile_pool(name="zg", bufs=2) as zgp,
                      tc.tile_pool(name="erg", bufs=2) as ergp,
                      tc.tile_pool(name="sp", bufs=2) as spp,
                      tc.tile_pool(name="exx", bufs=3) as exxp,
                      tc.tile_pool(name="msg", bufs=4) as msgp,
                      tc.tile_pool(name="p2s", bufs=4) as p2s,
                      tc.tile_pool(name="p2t", bufs=3) as p2t,
                      tc.tile_pool(name="msum", bufs=2, space="PSUM") as msump,
                      tc.tile_pool(name="eredge", bufs=2, space="PSUM") as erep,
                  ):
                      W16 = BL // 16
                      # one-time init of gather buffers: rows skipped by the
                      # runtime count would otherwise read uninitialized SBUF
                      # (NaN risk through exp -> matmul)
                      for i in range(2):
                          for r in range(2):
                              zg0 = zgp.tile([128, B, ZROW], BF16, tag=f"zg{r}")
                              nc.vector.memset(zg0[:], 0.0)
                      cregs = [[nc.gpsimd.alloc_register(f"cnt_w{w}_r{r}")
                                for r in range(2)] for w in range(NW)]
                      for w in range(NW):
                          nw = _win_nodes(w)
                          dwt = idxp.tile([128, 8], I16, tag="dw", bufs=2)
                          nc.sync.dma_start(
                              dwt[:], dw_in.ap().rearrange(
                                  "p (w c) -> p w c", c=8)[:, w, :])
                          msum = []
                          for r in range(2):
                              si = idxp.tile([128, W16], I16, tag=f"si{r}",
                                             bufs=2)
                              nc.sync.dma_start(
                                  si[:], si_in[r].ap().rearrange(
                                      "p (w c) -> p w c", c=W16)[:, w, :])
                              if DYN_GATHER:
                                  creg = cregs[w][r]
                                  nc.gpsimd.reg_load(
                                      creg, cnts[0:1, r * NW + w:r * NW + w + 1])
                                  nreg = creg
                              else:
                                  nreg = BL
                              zg = zgp.tile([128, B, ZROW], BF16, tag=f"zg{r}")
                              nc.gpsimd.dma_gather(
                                  zg[:], zpack[r][:], si[:],
                                  BL, nreg, ZROW, single_packet=False)
                              erw = ergp.tile([128, 1, ERROW], BF16,
                                              tag=f"erw{r}")
                              nc.gpsimd.dma_gather(
                                  erw[:], erd[r][:], dwt[:],
                                  128, 128, ERROW, single_packet=False)
                              ssb = spp.tile([128, BL], BF16, tag=f"ssb{r}")
                              nc.sync.dma_start(ssb[:], s_in[r][w])
                              stb = spp.tile([128, BL], BF16, tag=f"stb{r}")
                              nc.sync.dma_start(stb[:], st_in[r][w])
                              # er[dst] broadcast to edges via S^T matmul
                              ere = erep.tile([128, B, 64], F32, tag="ere")
                              for b in range(B):
                                  nc.tensor.matmul(
                                      ere[:, b, 0:T * H],
                                      stb[:, b * 128:(b + 1) * 128],
                                      erw[:, 0, 0:T * H],
                                      start=True, stop=True)
                              # e = el[src] + er[dst]; lk = leaky_relu(e)
                              el_ap = zg[:, :, 0:T * 68].rearrange(
                                  "p b (t c) -> p b t c", c=68)[:, :, :, 64:68]
                              lk = p2s.tile([128, B, T * H], BF16, tag="lk")
                              lk4 = lk[:].rearrange("p b (t h) -> p b t h", h=H)
                              nc.vector.tensor_add(
                                  lk4, el_ap,
                                  ere[:, :, 0:T * H].rearrange(
                                      "p b (t h) -> p b t h", h=H))
                              nc.vector.scalar_tensor_tensor(
                                  lk[:], lk[:], NEG_SLOPE, lk[:], ALU.mult, ALU.max)
                              if dump == "ere" and r == 0:
                                  dbg = p2t.tile([128, T * D], F32, tag="dbg")
                                  nc.vector.tensor_copy(
                                      dbg[:, 0:B * 48].rearrange(
                                          "p (b c) -> p b c", c=48),
                                      ere[:, :, 0:48])
                                  nc.vector.tensor_copy(dbg[:, 528:576],
                                                        erw[:, 0, 0:48])
                                  nc.scalar.activation(
                                      dbg[:, 576:624].rearrange(
                                          "p (t c) -> p t c", c=4),
                                      el_ap[:, 0], AF.Copy)
                                  nc.vector.memset(dbg[:, 624:768], 0.0)
                                  nc.sync.dma_start(
                                      out_d[w * WIN:w * WIN + nw], dbg[:nw])
                              exc = exxp.tile([128, B, T * H], BF16, tag="exc")
                              nc.scalar.activation(exc[:], lk[:], AF.Exp)
                              zap = zg[:, :, 0:T * 68].rearrange(
                                  "p b (t c) -> p b t c", c=68)[:, :, :, 0:64]
                              ms = msump.tile([128, 816], F32, tag="msum")
                              exc4 = exc[:].rearrange(
                                  "p b (t h) -> p b t h", h=H)
                              for b in range(B):
                                  msgb = msgp.tile([128, 816], BF16, tag="msg")
                                  nc.vector.tensor_mul(
                                      msgb[:, 0:768].rearrange(
                                          "p (t h k) -> p t h k", h=H, k=DH),
                                      zap[:, b].rearrange(
                                          "p t (h k) -> p t h k", k=DH),
                                      exc4[:, b].unsqueeze(3)
                                      .broadcast_to((128, T, H, DH)))
                                  nc.vector.tensor_copy(msgb[:, 768:816],
                                                        exc[:, b, :])
                                  lhsT = ssb[:, b * 128:(b + 1) * 128]
                                  nc.tensor.matmul(ms[:, 0:512], lhsT,
                                                   msgb[:, 0:512],
                                                   start=(b == 0), stop=(b == B - 1))
                                  nc.tensor.matmul(ms[:, 512:816], lhsT,
                                                   msgb[:, 512:816],
                                                   start=(b == 0), stop=(b == B - 1))
                              if dump == "msum" and r == 0:
                                  dbg = p2t.tile([128, T * D], F32, tag="dbg")
                                  nc.vector.tensor_copy(dbg[:, 0:48],
                                                        ms[:, 768:816])
                                  nc.vector.tensor_copy(dbg[:, 48:96],
                                                        exc[:, 0, :])
                                  nc.vector.tensor_copy(dbg[:, 96:768],
                                                        ms[:, 0:672])
                                  nc.sync.dma_start(
                                      out_d[w * WIN:w * WIN + nw], dbg[:nw])
                              msum.append(ms)
                          # epilogue: m = msgsum/denom (per rel), x2 = x + m1 + m2
                          xcw = p2t.tile([128, T * D], F32, tag="xcw")
                          nc.sync.dma_start(xcw[:nw], xc[w * WIN:w * WIN + nw])
                          x2w = x2p.tile([128, T * D], F32, tag="x2")
                          mtmp = p2t.tile([128, T * D], F32, tag="mtmp")
                          for r in range(2):
                              rec = p2s.tile([128, T * H], F32, tag="rec")
                              nc.vector.tensor_scalar_max(
                                  rec[:nw], msum[r][:nw, 768:816], 1e-16)
                              nc.vector.reciprocal(rec[:nw], rec[:nw])
                              rb = rec[:nw].rearrange(
                                  "p (t h) -> p t h", h=H).unsqueeze(3) \
                                  .broadcast_to((nw, T, H, DH))
                              dst = (x2w if r == 0 else mtmp)
                              nc.vector.tensor_mul(
                                  dst[:nw].rearrange(
                                      "p (t h k) -> p t h k", h=H, k=DH),
                                  msum[r][:nw, 0:768].rearrange(
                                      "p (t h k) -> p t h k", h=H, k=DH), rb)
                          nc.gpsimd.tensor_add(x2w[:nw], x2w[:nw], mtmp[:nw])
                          nc.gpsimd.tensor_add(x2w[:nw], x2w[:nw], xcw[:nw])
                          x2_tiles.append(x2w)
                          # BN2 stats only; scales computed batched in phase 3
                          st6 = p2s.tile([128, 2, 6], F32, tag="st6b")
                          nc.vector.bn_stats(st6[:nw, 0, :], x2w[:nw, 0:384])
                          nc.vector.bn_stats(st6[:nw, 1, :], x2w[:nw, 384:768])
                          nc.vector.bn_aggr(mvall[:nw, w, :], st6[:nw])

                  if phases < 3:
                      for w in range(NW):
                          nw = _win_nodes(w)
                          if dump is None:
                              nc.sync.dma_start(out_d[w * WIN:w * WIN + nw],
                                                x2_tiles[w][:nw])
                  else:
                    tc.strict_bb_all_engine_barrier()

                    # ---- Phase 3: BN2 apply + FFN + residual ----
                    with (
                        tc.tile_pool(name="p3", bufs=2) as p3,
                        tc.tile_pool(name="p3tp", bufs=1, space="PSUM") as p3tp,
                        tc.tile_pool(name="p3f1", bufs=2, space="PSUM") as p3f1,
                        tc.tile_pool(name="p3f2", bufs=1, space="PSUM") as p3f2,
                        tc.tile_pool(name="p3d", bufs=1, space="PSUM") as p3d,
                    ):
                        # batched BN2 scale: a = g*rsqrt(v+eps), b = beta - a*m
                        rsb = cpool.tile([128, NW], F32)
                        nc.scalar.activation(rsb[:], mvall[:, :, 1], AF.Sqrt,
                                             bias=epst[:])
                        nc.vector.reciprocal(rsb[:], rsb[:])
                        nc.vector.tensor_mul(aball[:, :, 0], gball[:, :, 0],
                                             rsb[:])
                        nc.vector.tensor_mul(aball[:, :, 1], aball[:, :, 0],
                                             mvall[:, :, 0])
                        nc.vector.tensor_sub(aball[:, :, 1], gball[:, :, 1],
                                             aball[:, :, 1])
                        for w in range(NW):
                            nw = _win_nodes(w)
                            x2w = x2_tiles[w]
                            h2 = p3.tile([128, T * D], BF16, tag="h2")
                            nc.scalar.activation(h2[:nw], x2w[:nw], AF.Identity,
                                                 bias=aball[:nw, w, 1:2],
                                                 scale=aball[:nw, w, 0:1])
                            tp = p3tp.tile([64, T, 128], BF16, tag="tp3")
                            for t in range(T):
                                nc.tensor.transpose(
                                    tp[:, t, 0:nw], h2[:nw, t * 64:(t + 1) * 64],
                                    ident[:nw, :nw])
                            h2t = p3.tile([64, T, 128], BF16, tag="h2t")
                            nc.vector.tensor_copy(h2t[:, :, 0:nw], tp[:, :, 0:nw])
                            if nw < 128:
                                nc.gpsimd.memset(h2t[:, :, nw:128], 0.0)
                            dd = p3d.tile([128, T, 64], BF16, tag="dd")
                            fft = p3.tile([64, T, 128], BF16, tag="fft")
                            for k in range(3):
                                rhs = h2t[:, 4 * k:4 * k + 4, :]
                                rhs = rhs.rearrange("p a b -> p (a b)")
                                f1 = p3f1.tile([128, 512], F32, tag="f1")
                                nc.tensor.matmul(f1[:], ffw1[0:64, :], rhs[:],
                                                 start=True, stop=True)
                                g1 = p3.tile([128, 512], BF16, tag="g1")
                                nc.scalar.activation(g1[:], f1[:], AF.Gelu,
                                                     bias=ffb1[:])
                                f2 = p3f2.tile([64, 512], F32, tag="f2")
                                nc.tensor.matmul(f2[:], ffw2[:], g1[:],
                                                 start=True, stop=True)
                                nc.scalar.activation(
                                    fft[:, 4 * k:4 * k + 4, :]
                                    .rearrange("p a b -> p (a b)"),
                                    f2[:], AF.Identity, bias=ffb2r[0:64, :])
                            for t in range(T):
                                nc.tensor.transpose(
                                    dd[0:nw, t, :], fft[:, t, 0:nw],
                                    ident[0:64, 0:64])
                            ot = p3.tile([128, T * D], F32, tag="ot")
                            nc.vector.tensor_add(
                                ot[:nw], dd[:nw].rearrange("p a b -> p (a b)"),
                                x2w[:nw])
                            nc.sync.dma_start(out_d[w * WIN:w * WIN + nw], ot[:nw])

    nc.compile()
    return nc


_CACHE = {}
_TRACE = False
_LAST_EXEC_NS = None


def _host_prep(inputs):
    x = np.asarray(inputs["x"], np.float32)
    xf = np.ascontiguousarray(x.reshape(N, T * D))
    B = 0
    for r in (1, 2):
        B = max(B, _max_blocks(np.asarray(inputs[f"src{r}"]),
                               np.asarray(inputs[f"dst{r}"])))

    bn1_gb = np.ascontiguousarray(
        np.stack([np.asarray(inputs["bn1_g"], np.float32),
                  np.asarray(inputs["bn1_b"], np.float32)], axis=1))
    bn2_gb_full = np.ascontiguousarray(
        np.stack([np.asarray(inputs["bn2_g"], np.float32),
                  np.asarray(inputs["bn2_b"], np.float32)], axis=1))
    common = {
        "x_full": xf,
        "bn1_gb": bn1_gb,
        "ffw1": np.ascontiguousarray(np.asarray(inputs["ff_w1"], np.float32)),
        "ffb1": np.ascontiguousarray(
            np.asarray(inputs["ff_b1"], np.float32).reshape(DFF, 1)),
        "ffw2": np.ascontiguousarray(np.asarray(inputs["ff_w2"], np.float32)),
        "ffb2": np.ascontiguousarray(
            np.asarray(inputs["ff_b2"], np.float32).reshape(D, 1)),
        "ident": np.eye(128, dtype=BF16NP),
    }
    for r in (1, 2):
        W = np.asarray(inputs[f"W{r}"], np.float32).reshape(D, H * DH)
        al = np.asarray(inputs[f"al{r}"], np.float32).reshape(-1)
        ar = np.asarray(inputs[f"ar{r}"], np.float32).reshape(-1)
        common[f"W{r}"] = np.ascontiguousarray(W)
        common[f"al{r}t"] = np.ascontiguousarray(np.tile(al[None, :], (D, 1)))
        common[f"ar{r}t"] = np.ascontiguousarray(np.tile(ar[None, :], (D, 1)))

    in_maps = []
    for m in range(NCORES):
        lo = m * CHUNK
        im = dict(common)
        im["xc"] = np.ascontiguousarray(xf[lo:lo + CHUNK])
        bn2p = np.zeros((NW * 128, 2), np.float32)
        bn2p[:CHUNK] = bn2_gb_full[lo:lo + CHUNK]
        im["bn2_gb"] = np.ascontiguousarray(bn2p)
        # dst-window node ids (for the per-window er gather); padded with 0
        dwin = np.zeros((NW, 128), np.int16)
        for w in range(NW):
            nw = _win_nodes(w)
            dwin[w, :nw] = np.arange(lo + w * WIN, lo + w * WIN + nw)
        im["dstwin"] = _wrap16(dwin.reshape(-1)).reshape(128, NW * 8)
        cnt_all = np.zeros((1, 2 * NW), np.int32)
        for r in (1, 2):
            src16, cnts, S, ST = _prep_core_rel(
                np.asarray(inputs[f"src{r}"]), np.asarray(inputs[f"dst{r}"]),
                lo, B)
            im[f"srcidx{r}"] = src16
            im[f"S{r}"] = S
            im[f"ST{r}"] = ST
            cnt_all[0, (r - 1) * NW:r * NW] = cnts
        im["cnts"] = cnt_all
        in_maps.append(im)
    return B, in_maps


def kernel(**inputs):
    B, in_maps = _host_prep(inputs)
    if B not in _CACHE:
        _CACHE[B] = _build_program(B)
    nc = _CACHE[B]
    global _LAST_EXEC_NS
    res = run_bass_kernel_spmd(nc, in_maps, core_ids=list(range(NCORES)),
                               trace=_TRACE)
    _LAST_EXEC_NS = res.exec_time_ns
    out = np.concatenate([res.results[m]["OUT"] for m in range(NCORES)], axis=0)
    return out.reshape(N, T, D).astype(np.float32)


# revision 19
# speedup vs baseline: 1.1975x; 1.1975x over previous
"""Trainium2 Bass kernel for nn_EncoderLayer (GNN message passing, 2-relation GAT).

Sharding: nodes (and incoming-edge lists, partitioned by dst) sharded across 8
cores; small GAT/FFN weights replicated; gathered src features fetched from a
replicated projection table via indexed DMA (dma_gather).

Per-core device program:
  Phase 0: cast/fold weights (W_ext = [W | W.al | W.ar]) on device.
  Phase 1: BN1 + z/el/er projection for ALL nodes (replicated), packed rows
           written to DRAM: Zpack[node] = 12 x (64 z | 4 el) bf16, Er[node].
           BN rsqrt via scalar Sqrt + vector reciprocal (keeps scalar on one
           activation table for the whole phase).
  Phase 2: per dst-window (128 nodes): dma_gather Zpack[src] with a per-core
           runtime row count (tail indices are -1 and skipped); er[dst] comes
           from a 128-row window gather broadcast to edges via an S^T matmul;
           ex = exp(leaky(el+er)); msg = ex*z; segment-sum via one-hot
           S-matrix matmuls accumulating in PSUM (denominator columns fed
           straight from the ex tile); m = msgsum/denom; x2 = x + m1 + m2;
           BN2 stats stashed per window.
  Phase 3: batched BN2 scale computation (one Sqrt), then per window:
           h2 = BN2(x2); FFN via PE transposes; out = x2 + FFN.
"""

import sys

sys.path.insert(0, "/opt/trn_rl_repo")

import numpy as np
import ml_dtypes

import concourse.bass as bass
import concourse.bacc as bacc
import concourse.tile as tile
import concourse.mybir as mybir
from concourse.bass_utils import run_bass_kernel_spmd

F32 = mybir.dt.float32
BF16 = mybir.dt.bfloat16
I16 = mybir.dt.int16
I32 = mybir.dt.int32
AF = mybir.ActivationFunctionType
ALU = mybir.AluOpType
BF16NP = ml_dtypes.bfloat16

N, T, D, H, DH, DFF = 10000, 12, 64, 4, 16, 128
NCORES = 8
CHUNK = N // NCORES          # 1250
WIN = 128                    # dst-window size (nodes)
NW = (CHUNK + WIN - 1) // WIN  # 10 windows; last has 98 nodes
EPS = 1e-5
NEG_SLOPE = 0.2
ZROW = 896                   # padded Zpack row (bf16 elems): 12*68 data + 80 pad
ERROW = 128                  # padded Er row (bf16 elems): 48 data + 80 pad
NBLK = (N + 127) // 128      # 79 phase-1 blocks (last = 16 nodes)
DYN_GATHER = True            # runtime row counts (-1 padded) for zg gathers


def _win_nodes(w):
    return min(WIN, CHUNK - w * WIN)


def _wrap16(arr):
    """dma_gather index layout: idx i at [i%16, i//16], tiled to 128 parts."""
    return np.ascontiguousarray(np.tile(arr.reshape(-1, 16).T, (8, 1)))


def _prep_core_rel(src, dst, lo, B):
    """Edge lists for one (core, relation): sorted by dst, windowed, padded to
    B blocks of 128 edges per window with -1 (skipped by dma_gather).
    Returns (src16, cnts, S, ST) where cnts[w] is the real edge count and
    S[w, edge_in_block, block*128 + local_node] is the one-hot segment-sum
    stationary (ST its per-block transpose)."""
    hi = lo + CHUNK
    sel = (dst >= lo) & (dst < hi)
    es = src[sel].astype(np.int64)
    ed = (dst[sel] - lo).astype(np.int64)
    order = np.argsort(ed, kind="stable")
    es, ed = es[order], ed[order]
    L = NW * B * 128
    src_arr = np.full(L, -1 if DYN_GATHER else 0, np.int16)
    cnts = np.zeros(NW, np.int32)
    S = np.zeros((NW, 128, B * 128), BF16NP)
    ST = np.zeros((NW, 128, B * 128), BF16NP)
    wstart = np.searchsorted(ed, np.arange(NW) * WIN)
    wend = np.searchsorted(ed, np.arange(1, NW + 1) * WIN)
    for w in range(NW):
        seg_src = es[wstart[w]:wend[w]]
        seg_dst = ed[wstart[w]:wend[w]]
        cnt = len(seg_src)
        assert 0 < cnt <= B * 128
        base = w * B * 128
        src_arr[base:base + cnt] = seg_src
        cnts[w] = cnt
        i = np.arange(cnt)
        S[w, i % 128, (i // 128) * 128 + (seg_dst - w * WIN)] = 1.0
        ST[w, seg_dst - w * WIN, (i // 128) * 128 + (i % 128)] = 1.0
    return _wrap16(src_arr), cnts, S, ST


def _max_blocks(src, dst):
    """Max number of 128-edge blocks any (core, window) needs for this rel."""
    best = 0
    for m in range(NCORES):
        lo = m * CHUNK
        sel = (dst >= lo) & (dst < lo + CHUNK)
        ed = dst[sel] - lo
        cnt = np.bincount(ed // WIN, minlength=NW)
        best = max(best, int(np.max((cnt + 127) // 128)))
    return best


def _build_program(B, phases=3, dump=None):
    nc = bacc.Bacc("TRN2", target_bir_lowering=False, debug=False,
                   num_devices=NCORES)
    BL = B * 128              # edges per window (padded)
    L = NW * BL               # edges per (core, rel)
    L16 = L // 16

    # ---- DRAM tensors ----
    x_full = nc.dram_tensor("x_full", [N, T * D], F32, kind="ExternalInput")
    xc = nc.dram_tensor("xc", [CHUNK, T * D], F32, kind="ExternalInput")
    bn1_gb = nc.dram_tensor("bn1_gb", [N, 2], F32, kind="ExternalInput")
    bn2_gb = nc.dram_tensor("bn2_gb", [NW * 128, 2], F32, kind="ExternalInput")
    cnt_in = nc.dram_tensor("cnts", [1, 2 * NW], I32, kind="ExternalInput")
    dw_in = nc.dram_tensor("dstwin", [128, NW * 8], I16, kind="ExternalInput")
    w_in, al_in, ar_in, s_in, st_in, si_in = [], [], [], [], [], []
    for r in (1, 2):
        w_in.append(nc.dram_tensor(f"W{r}", [D, H * DH], F32, kind="ExternalInput"))
        al_in.append(nc.dram_tensor(f"al{r}t", [D, H * DH], F32, kind="ExternalInput"))
        ar_in.append(nc.dram_tensor(f"ar{r}t", [D, H * DH], F32, kind="ExternalInput"))
        s_in.append(nc.dram_tensor(f"S{r}", [NW, 128, BL], BF16, kind="ExternalInput"))
        st_in.append(nc.dram_tensor(f"ST{r}", [NW, 128, BL], BF16, kind="ExternalInput"))
        si_in.append(nc.dram_tensor(f"srcidx{r}", [128, L16], I16, kind="ExternalInput"))
    ffw1_in = nc.dram_tensor("ffw1", [D, DFF], F32, kind="ExternalInput")
    ffb1_in = nc.dram_tensor("ffb1", [DFF, 1], F32, kind="ExternalInput")
    ffw2_in = nc.dram_tensor("ffw2", [DFF, D], F32, kind="ExternalInput")
    ffb2_in = nc.dram_tensor("ffb2", [D, 1], F32, kind="ExternalInput")
    ident_in = nc.dram_tensor("ident", [128, 128], BF16, kind="ExternalInput")
    out_d = nc.dram_tensor("OUT", [CHUNK, T * D], F32, kind="ExternalOutput")

    zpack = [nc.dram_tensor(f"zpack{r}", [N, ZROW], BF16, kind="Internal")
             for r in (1, 2)]
    erd = [nc.dram_tensor(f"er{r}", [N, ERROW], BF16, kind="Internal")
           for r in (1, 2)]

    with tile.TileContext(nc) as tc:
        with tc.tile_pool(name="const", bufs=1) as cpool:
            # ---- Phase 0: weights to SBUF, fold al/ar into W_ext ----
            ident = cpool.tile([128, 128], BF16)
            nc.sync.dma_start(ident[:], ident_in[:])
            cnts = cpool.tile([1, 2 * NW], I32)
            nc.sync.dma_start(cnts[:], cnt_in[:])
            wext = []
            for r in range(2):
                wf = cpool.tile([D, H * DH], F32, tag="wf")
                nc.sync.dma_start(wf[:], w_in[r][:])
                we = cpool.tile([128, 72], BF16, tag=f"wext{r}")
                nc.vector.tensor_copy(we[0:D, 0:64], wf[:])
                for k, t_in in ((64, al_in[r]), (68, ar_in[r])):
                    alt = cpool.tile([D, H * DH], F32, tag="alt")
                    nc.sync.dma_start(alt[:], t_in[:])
                    prod = cpool.tile([D, H * DH], F32, tag="prod")
                    nc.vector.tensor_mul(prod[:], wf[:], alt[:])
                    red = cpool.tile([D, H], F32, tag="red")
                    nc.vector.tensor_reduce(
                        red[:].unsqueeze(2),
                        prod[:].rearrange("p (h k) -> p h k", k=DH),
                        mybir.AxisListType.X, ALU.add)
                    nc.vector.tensor_copy(we[0:D, k:k + H], red[:])
                wext.append(we)
            wboth = cpool.tile([64, 144], BF16)
            for r in range(2):
                nc.vector.tensor_copy(wboth[0:D, 72 * r:72 * r + 72],
                                      wext[r][0:D, :])
            ffw1 = cpool.tile([128, DFF], BF16)
            t1 = cpool.tile([D, DFF], F32, tag="t1")
            nc.sync.dma_start(t1[:], ffw1_in[:])
            nc.vector.tensor_copy(ffw1[0:D, :], t1[:])
            nc.sync.dma_start(ffw1[64:128, :], ffw1[0:64, :])
            ffw2 = cpool.tile([DFF, D], BF16)
            t2 = cpool.tile([DFF, D], F32, tag="t2")
            nc.sync.dma_start(t2[:], ffw2_in[:])
            nc.vector.tensor_copy(ffw2[:], t2[:])
            ffb1 = cpool.tile([DFF, 1], F32)
            nc.sync.dma_start(ffb1[:], ffb1_in[:])
            ffb2r = cpool.tile([128, 1], F32)
            nc.sync.dma_start(ffb2r[0:64, :], ffb2_in[:])
            nc.sync.dma_start(ffb2r[64:128, :], ffb2_in[:])
            epst = cpool.tile([128, 1], F32)
            nc.vector.memset(epst[:], EPS)
            # per-window BN2 stats/params stash (consumed batched in phase 3)
            mvall = cpool.tile([128, NW, 2], F32)
            gball = cpool.tile([128, NW, 2], F32)
            nc.sync.dma_start(
                gball[:],
                bn2_gb.ap().rearrange("(w p) c -> p w c", p=128))
            aball = cpool.tile([128, NW, 2], F32)

            if phases >= 1:
              # ---- Phase 1: BN1 + projections for all N nodes ----
              with (
                  tc.tile_pool(name="p1", bufs=3) as p1,
                  tc.tile_pool(name="p1s", bufs=3) as p1s,
                  tc.tile_pool(name="p1tp", bufs=1, space="PSUM") as p1tp,
                  tc.tile_pool(name="p1zp", bufs=1, space="PSUM") as p1zp,
              ):
                  for blk in range(NBLK):
                      n0 = blk * 128
                      nb = min(128, N - n0)
                      xt = p1.tile([128, T * D], F32, tag="xt")
                      nc.sync.dma_start(xt[:nb], x_full[n0:n0 + nb])
                      st6 = p1s.tile([128, 2, 6], F32, tag="st6")
                      nc.vector.bn_stats(st6[:nb, 0, :], xt[:nb, 0:384])
                      nc.vector.bn_stats(st6[:nb, 1, :], xt[:nb, 384:768])
                      mv = p1s.tile([128, 2], F32, tag="mv")
                      nc.vector.bn_aggr(mv[:nb], st6[:nb])
                      gb = p1s.tile([128, 2], F32, tag="gb")
                      nc.sync.dma_start(gb[:nb], bn1_gb[n0:n0 + nb])
                      rs = p1s.tile([128, 1], F32, tag="rs")
                      nc.scalar.activation(rs[:nb], mv[:nb, 1:2], AF.Sqrt,
                                           bias=epst[:nb])
                      nc.vector.reciprocal(rs[:nb], rs[:nb])
                      a = p1s.tile([128, 1], F32, tag="a")
                      nc.vector.tensor_mul(a[:nb], gb[:nb, 0:1], rs[:nb])
                      b = p1s.tile([128, 1], F32, tag="b")
                      nc.vector.tensor_mul(b[:nb], a[:nb], mv[:nb, 0:1])
                      nc.vector.tensor_sub(b[:nb], gb[:nb, 1:2], b[:nb])
                      h = p1.tile([128, T * D], BF16, tag="h")
                      nc.scalar.activation(h[:nb], xt[:nb], AF.Identity,
                                           bias=b[:nb], scale=a[:nb])
                      tp = p1tp.tile([64, T, 128], BF16, tag="tp")
                      for t in range(T):
                          nc.tensor.transpose(
                              tp[:, t, 0:nb], h[:nb, t * 64:(t + 1) * 64],
                              ident[:nb, :nb])
                      ht = p1.tile([64, T, 128], BF16, tag="ht")
                      nc.vector.tensor_copy(ht[:, :, 0:nb], tp[:, :, 0:nb])
                      zp = [p1zp.tile([128, T, 128], F32, tag=f"zp{r}",
                                      name=f"zp{r}") for r in range(2)]
                      for t in range(T):
                          lhsT = ht[:, t, 0:nb]
                          for r in range(2):
                              nc.tensor.matmul(
                                  zp[r][0:nb, t, 0:72], lhsT,
                                  wext[r][0:64, :],
                                  start=True, stop=True)
                      for r in range(2):
                          zel = p1.tile([128, ZROW], BF16, tag=f"zel{r}")
                          nc.scalar.activation(
                              zel[:nb, 0:T * 68].rearrange("p (t c) -> p t c", t=T),
                              zp[r][:nb, :, 0:68], AF.Copy)
                          ers = p1s.tile([128, ERROW], BF16, tag=f"ers{r}")
                          nc.scalar.activation(
                              ers[:nb, 0:T * H].rearrange("p (t c) -> p t c", t=T),
                              zp[r][:nb, :, 68:72], AF.Copy)
                          nc.sync.dma_start(zpack[r][n0:n0 + nb, :], zel[:nb])
                          nc.sync.dma_start(erd[r][n0:n0 + nb, :], ers[:nb])

            tc.strict_bb_all_engine_barrier()

            if phases < 2:
                dummy = cpool.tile([128, 16], F32)
                nc.vector.memset(dummy[:], 0.0)
                for w in range(NW):
                    nw = _win_nodes(w)
                    nc.sync.dma_start(out_d[w * WIN:w * WIN + nw, 0:16],
                                      dummy[:nw])
            if phases >= 2:
              # ---- Phase 2: gather + attention + segment-sum + BN2 stats ----
              x2_tiles = []
              with (
                  tc.tile_pool(name="x2p", bufs=NW) as x2p,
              ):
                  with (
                      tc.tile_pool(name="idxp", bufs=1) as idxp,
                      tc.tile_pool(name="zg", bufs=2) as zgp,
                      tc.tile_pool(name="erg", bufs=2) as ergp,
                      tc.tile_pool(name="sp", bufs=2) as spp,
                      tc.tile_pool(name="exx", bufs=3) as exxp,
                      tc.tile_pool(name="msg", bufs=4) as msgp,
                      tc.tile_pool(name="p2s", bufs=4) as p2s,
                      tc.tile_pool(name="p2t", bufs=3) as p2t,
                      tc.tile_pool(name="msum", bufs=2, space="PSUM") as msump,
                      tc.tile_pool(name="eredge", bufs=2, space="PSUM") as erep,
                  ):
                      W16 = BL // 16
                      # one-time init of gather buffers: rows skipped by the
                      # runtime count would otherwise read uninitialized SBUF
                      # (NaN risk through exp -> matmul)
                      for i in range(2):
                          for r in range(2):
                              zg0 = zgp.tile([128, B, ZROW], BF16, tag=f"zg{r}")
                              nc.vector.memset(zg0[:], 0.0)
                      cregs = [[nc.gpsimd.alloc_register(f"cnt_w{w}_r{r}")
                                for r in range(2)] for w in range(NW)]
                      for w in range(NW):
                          nw = _win_nodes(w)
                          dwt = idxp.tile([128, 8], I16, tag="dw", bufs=2)
                          nc.sync.dma_start(
                              dwt[:], dw_in.ap().rearrange(
                                  "p (w c) -> p w c", c=8)[:, w, :])
                          msum = []
                          for r in range(2):
                              si = idxp.tile([128, W16], I16, tag=f"si{r}",
                                             bufs=2)
                              nc.sync.dma_start(
                                  si[:], si_in[r].ap().rearrange(
                                      "p (w c) -> p w c", c=W16)[:, w, :])
                              if DYN_GATHER:
                                  creg = cregs[w][r]
                                  nc.gpsimd.reg_load(
                                      creg, cnts[0:1, r * NW + w:r * NW + w + 1])
                                  nreg = creg
                              else:
                                  nreg = BL
                              zg = zgp.tile([128, B, ZROW], BF16, tag=f"zg{r}")
                              nc.gpsimd.dma_gather(
                                  zg[:], zpack[r][:], si[:],
                                  BL, nreg, ZROW, single_packet=False)
                              erw = ergp.tile([128, 1, ERROW], BF16,
                                              tag=f"erw{r}")
                              nc.gpsimd.dma_gather(
                                  erw[:], erd[r][:], dwt[:],
                                  128, 128, ERROW, single_packet=False)
                              ssb = spp.tile([128, BL], BF16, tag=f"ssb{r}")
                              nc.sync.dma_start(ssb[:], s_in[r][w])
                              stb = spp.tile([128, BL], BF16, tag=f"stb{r}")
                              nc.sync.dma_start(stb[:], st_in[r][w])
                              # er[dst] broadcast to edges via S^T matmul
                              ere = erep.tile([128, B, 64], F32, tag="ere")
                              for b in range(B):
                                  nc.tensor.matmul(
                                      ere[:, b, 0:T * H],
                                      stb[:, b * 128:(b + 1) * 128],
                                      erw[:, 0, 0:T * H],
                                      start=True, stop=True)
                              # e = el[src] + er[dst]; lk = leaky_relu(e)
                              el_ap = zg[:, :, 0:T * 68].rearrange(
                                  "p b (t c) -> p b t c", c=68)[:, :, :, 64:68]
                              lk = p2s.tile([128, B, T * H], BF16, tag="lk")
                              lk4 = lk[:].rearrange("p b (t h) -> p b t h", h=H)
                              nc.vector.tensor_add(
                                  lk4, el_ap,
                                  ere[:, :, 0:T * H].rearrange(
                                      "p b (t h) -> p b t h", h=H))
                              nc.vector.scalar_tensor_tensor(
                                  lk[:], lk[:], NEG_SLOPE, lk[:], ALU.mult, ALU.max)
                              if dump == "ere" and r == 0:
                                  dbg = p2t.tile([128, T * D], F32, tag="dbg")
                                  nc.vector.tensor_copy(
                                      dbg[:, 0:B * 48].rearrange(
                                          "p (b c) -> p b c", c=48),
                                      ere[:, :, 0:48])
                                  nc.vector.tensor_copy(dbg[:, 528:576],
                                                        erw[:, 0, 0:48])
                                  nc.scalar.activation(
                                      dbg[:, 576:624].rearrange(
                                          "p (t c) -> p t c", c=4),
                                      el_ap[:, 0], AF.Copy)
                                  nc.vector.memset(dbg[:, 624:768], 0.0)
                                  nc.sync.dma_start(
                                      out_d[w * WIN:w * WIN + nw], dbg[:nw])
                              exc = exxp.tile([128, B, T * H], BF16, tag="exc")
                              nc.scalar.activation(exc[:], lk[:], AF.Exp)
                              zap = zg[:, :, 0:T * 68].rearrange(
                                  "p b (t c) -> p b t c", c=68)[:, :, :, 0:64]
                              ms = msump.tile([128, 816], F32, tag="msum")
                              exc4 = exc[:].rearrange(
                                  "p b (t h) -> p b t h", h=H)
                              for b in range(B):
                                  msgb = msgp.tile([128, 816], BF16, tag="msg")
                                  nc.vector.tensor_mul(
                                      msgb[:, 0:768].rearrange(
                                          "p (t h k) -> p t h k", h=H, k=DH),
                                      zap[:, b].rearrange(
                                          "p t (h k) -> p t h k", k=DH),
                                      exc4[:, b].unsqueeze(3)
                                      .broadcast_to((128, T, H, DH)))
                                  nc.vector.tensor_copy(msgb[:, 768:816],
                                                        exc[:, b, :])
                                  lhsT = ssb[:, b * 128:(b + 1) * 128]
                                  nc.tensor.matmul(ms[:, 0:512], lhsT,
                                                   msgb[:, 0:512],
                                                   start=(b == 0), stop=(b == B - 1))
                                  nc.tensor.matmul(ms[:, 512:816], lhsT,
                                                   msgb[:, 512:816],
                                                   start=(b == 0), stop=(b == B - 1))
                              if dump == "msum" and r == 0:
                                  dbg = p2t.tile([128, T * D], F32, tag="dbg")
                                  nc.vector.tensor_copy(dbg[:, 0:48],
                                                        ms[:, 768:816])
                                  nc.vector.tensor_copy(dbg[:, 48:96],
                                                        exc[:, 0, :])
                                  nc.vector.tensor_copy(dbg[:, 96:768],
                                                        ms[:, 0:672])
                                  nc.sync.dma_start(
                                      out_d[w * WIN:w * WIN + nw], dbg[:nw])
                              msum.append(ms)
                          # epilogue: m = msgsum/denom (per rel), x2 = x + m1 + m2
                          xcw = p2t.tile([128, T * D], F32, tag="xcw")
                          nc.sync.dma_start(xcw[:nw], xc[w * WIN:w * WIN + nw])
                          x2w = x2p.tile([128, T * D], F32, tag="x2")
                          mtmp = p2t.tile([128, T * D], F32, tag="mtmp")
                          for r in range(2):
                              rec = p2s.tile([128, T * H], F32, tag="rec")
                              nc.vector.tensor_scalar_max(
                                  rec[:nw], msum[r][:nw, 768:816], 1e-16)
                              nc.vector.reciprocal(rec[:nw], rec[:nw])
                              rb = rec[:nw].rearrange(
                                  "p (t h) -> p t h", h=H).unsqueeze(3) \
                                  .broadcast_to((nw, T, H, DH))
                              dst = (x2w if r == 0 else mtmp)
                              nc.vector.tensor_mul(
                                  dst[:nw].rearrange(
                                      "p (t h k) -> p t h k", h=H, k=DH),
                                  msum[r][:nw, 0:768].rearrange(
                                      "p (t h k) -> p t h k", h=H, k=DH), rb)
                          nc.gpsimd.tensor_add(x2w[:nw], x2w[:nw], mtmp[:nw])
                          nc.gpsimd.tensor_add(x2w[:nw], x2w[:nw], xcw[:nw])
                          x2_tiles.append(x2w)
                          # BN2 stats only; scales computed batched in phase 3
                          st6 = p2s.tile([128, 2, 6], F32, tag="st6b")
                          nc.vector.bn_stats(st6[:nw, 0, :], x2w[:nw, 0:384])
                          nc.vector.bn_stats(st6[:nw, 1, :], x2w[:nw, 384:768])
                          nc.vector.bn_aggr(mvall[:nw, w, :], st6[:nw])

                  if phases < 3:
                      for w in range(NW):
                          nw = _win_nodes(w)
                          if dump is None:
                              nc.sync.dma_start(out_d[w * WIN:w * WIN + nw],
                                                x2_tiles[w][:nw])
                  else:
                    tc.strict_bb_all_engine_barrier()

                    # ---- Phase 3: BN2 apply + FFN + residual ----
                    with (
                        tc.tile_pool(name="p3", bufs=2) as p3,
                        tc.tile_pool(name="p3tp", bufs=1, space="PSUM") as p3tp,
                        tc.tile_pool(name="p3f1", bufs=2, space="PSUM") as p3f1,
                        tc.tile_pool(name="p3f2", bufs=1, space="PSUM") as p3f2,
                        tc.tile_pool(name="p3d", bufs=1, space="PSUM") as p3d,
                    ):
                        # batched BN2 scale: a = g*rsqrt(v+eps), b = beta - a*m
                        rsb = cpool.tile([128, NW], F32)
                        nc.scalar.activation(rsb[:], mvall[:, :, 1], AF.Sqrt,
                                             bias=epst[:])
                        nc.vector.reciprocal(rsb[:], rsb[:])
                        nc.vector.tensor_mul(aball[:, :, 0], gball[:, :, 0],
                                             rsb[:])
                        nc.vector.tensor_mul(aball[:, :, 1], aball[:, :, 0],
                                             mvall[:, :, 0])
                        nc.vector.tensor_sub(aball[:, :, 1], gball[:, :, 1],
                                             aball[:, :, 1])
                        for w in range(NW):
                            nw = _win_nodes(w)
                            x2w = x2_tiles[w]
                            h2 = p3.tile([128, T * D], BF16, tag="h2")
                            nc.scalar.activation(h2[:nw], x2w[:nw], AF.Identity,
                                                 bias=aball[:nw, w, 1:2],
                                                 scale=aball[:nw, w, 0:1])
                            tp = p3tp.tile([64, T, 128], BF16, tag="tp3")
                            for t in range(T):
                                nc.tensor.transpose(
                                    tp[:, t, 0:nw], h2[:nw, t * 64:(t + 1) * 64],
                                    ident[:nw, :nw])
                            h2t = p3.tile([64, T, 128], BF16, tag="h2t")
                            nc.vector.tensor_copy(h2t[:, :, 0:nw], tp[:, :, 0:nw])
                            if nw < 128:
                                nc.gpsimd.memset(h2t[:, :, nw:128], 0.0)
                            dd = p3d.tile([128, T, 64], BF16, tag="dd")
                            fft = p3.tile([64, T, 128], BF16, tag="fft")
                            for k in range(3):
                                rhs = h2t[:, 4 * k:4 * k + 4, :]
                                rhs = rhs.rearrange("p a b -> p (a b)")
                                f1 = p3f1.tile([128, 512], F32, tag="f1")
                                nc.tensor.matmul(f1[:], ffw1[0:64, :], rhs[:],
                                                 start=True, stop=True)
                                g1 = p3.tile([128, 512], BF16, tag="g1")
                                nc.scalar.activation(g1[:], f1[:], AF.Gelu,
                                                     bias=ffb1[:])
                                f2 = p3f2.tile([64, 512], F32, tag="f2")
                                nc.tensor.matmul(f2[:], ffw2[:], g1[:],
                                                 start=True, stop=True)
                                nc.scalar.activation(
                                    fft[:, 4 * k:4 * k + 4, :]
                                    .rearrange("p a b -> p (a b)"),
                                    f2[:], AF.Identity, bias=ffb2r[0:64, :])
                            for t in range(T):
                                nc.tensor.transpose(
                                    dd[0:nw, t, :], fft[:, t, 0:nw],
                                    ident[0:64, 0:64])
                            ot = p3.tile([128, T * D], F32, tag="ot")
                            nc.vector.tensor_add(
                                ot[:nw], dd[:nw].rearrange("p a b -> p (a b)"),
                                x2w[:nw])
                            nc.sync.dma_start(out_d[w * WIN:w * WIN + nw], ot[:nw])

    nc.compile()
    return nc


_CACHE = {}
_TRACE = False
_LAST_EXEC_NS = None


def _host_prep(inputs):
    x = np.asarray(inputs["x"], np.float32)
    xf = np.ascontiguousarray(x.reshape(N, T * D))
    B = 0
    for r in (1, 2):
        B = max(B, _max_blocks(np.asarray(inputs[f"src{r}"]),
                               np.asarray(inputs[f"dst{r}"])))

    bn1_gb = np.ascontiguousarray(
        np.stack([np.asarray(inputs["bn1_g"], np.float32),
                  np.asarray(inputs["bn1_b"], np.float32)], axis=1))
    bn2_gb_full = np.ascontiguousarray(
        np.stack([np.asarray(inputs["bn2_g"], np.float32),
                  np.asarray(inputs["bn2_b"], np.float32)], axis=1))
    common = {
        "x_full": xf,
        "bn1_gb": bn1_gb,
        "ffw1": np.ascontiguousarray(np.asarray(inputs["ff_w1"], np.float32)),
        "ffb1": np.ascontiguousarray(
            np.asarray(inputs["ff_b1"], np.float32).reshape(DFF, 1)),
        "ffw2": np.ascontiguousarray(np.asarray(inputs["ff_w2"], np.float32)),
        "ffb2": np.ascontiguousarray(
            np.asarray(inputs["ff_b2"], np.float32).reshape(D, 1)),
        "ident": np.eye(128, dtype=BF16NP),
    }
    for r in (1, 2):
        W = np.asarray(inputs[f"W{r}"], np.float32).reshape(D, H * DH)
        al = np.asarray(inputs[f"al{r}"], np.float32).reshape(-1)
        ar = np.asarray(inputs[f"ar{r}"], np.float32).reshape(-1)
        common[f"W{r}"] = np.ascontiguousarray(W)
        common[f"al{r}t"] = np.ascontiguousarray(np.tile(al[None, :], (D, 1)))
        common[f"ar{r}t"] = np.ascontiguousarray(np.tile(ar[None, :], (D, 1)))

    in_maps = []
    for m in range(NCORES):
        lo = m * CHUNK
        im = dict(common)
        im["xc"] = np.ascontiguousarray(xf[lo:lo + CHUNK])
        bn2p = np.zeros((NW * 128, 2), np.float32)
        bn2p[:CHUNK] = bn2_gb_full[lo:lo + CHUNK]
        im["bn2_gb"] = np.ascontiguousarray(bn2p)
        # dst-window node ids (for the per-window er gather); padded with 0
        dwin = np.zeros((NW, 128), np.int16)
        for w in range(NW):
            nw = _win_nodes(w)
            dwin[w, :nw] = np.arange(lo + w * WIN, lo + w * WIN + nw)
        im["dstwin"] = _wrap16(dwin.reshape(-1)).reshape(128, NW * 8)
        cnt_all = np.zeros((1, 2 * NW), np.int32)
        for r in (1, 2):
            src16, cnts, S, ST = _prep_core_rel(
                np.asarray(inputs[f"src{r}"]), np.asarray(inputs[f"dst{r}"]),
                lo, B)
            im[f"srcidx{r}"] = src16
            im[f"S{r}"] = S
            im[f"ST{r}"] = ST
            cnt_all[0, (r - 1) * NW:r * NW] = cnts
        im["cnts"] = cnt_all
        in_maps.append(im)
    return B, in_maps


def kernel(**inputs):
    B, in_maps = _host_prep(inputs)
    if B not in _CACHE:
        _CACHE[B] = _build_program(B)
    nc = _CACHE[B]
    global _LAST_EXEC_NS
    res = run_bass_kernel_spmd(nc, in_maps, core_ids=list(range(NCORES)),
                               trace=_TRACE)
    _LAST_EXEC_NS = res.exec_time_ns
    out = np.concatenate([res.results[m]["OUT"] for m in range(NCORES)], axis=0)
    return out.reshape(N, T, D).astype(np.float32)
